# revision 4
# baseline (speedup 1.0000x reference)
"""Fused multi-head attention (B=4, N=2048, C=1024, H=16) for 8 trn2 NeuronCores.

Sharding: batch x head-half hybrid. Core c owns batch b = c>>1 and head-half
hh = c&1 (8 heads = channel dims hh*512..hh*512+512, as 4 head-pairs). Each
core computes QKV for its batch restricted to its 512 dims, attention for its
8 heads, and the partial output projection [2048, 1024] for its batch; the
host sums the 2 partials per batch and adds bo.

Head-pair-major fused schedule (v2): the old kernel ran QKV fully (PE-dense,
ACT/DVE idle) then attention (exp-bound on ACT/DVE, PE ~15% idle). Now only
V + the first head-pair's Q/K run up front; each attention window for head
pair hp drips the NEXT pair's Q/K projection matmuls (one per step) into the
PE slack that the exp latency leaves, and the last window drips the output
projection. This hides ~55us of projection work inside the exp-bound phase.

Per step (qb, hp, kt): the two heads' score matmuls run as row-tiled
CONCURRENT matmuls (K=64 each, tile_position (0,0)/(64,0)) into one
2-bank PSUM tile [128, 2, 512]; ONE merged exp instruction covers both
heads (alternating: even kt -> DVE Schraudolph fp16-bit exp, odd kt -> ACT
exact exp), halving per-instruction overheads; AV trails by AV_LAG steps.
Softmax denominators come from ones-columns in the packed V blocks
([V_h0|ones|V_h1|ones] per k-tile). Normalization (reciprocal + DMA
broadcast of 1/den + multiply into CT) is emitted DEFERRED by a few steps
so the DMA-roundtrip latency never head-of-line-blocks the DVE/ACT queues.
"""

import os
import sys

import numpy as np

if not os.path.isdir(os.path.join(os.path.dirname(os.path.abspath(__file__)), "concourse")):
    for _p in ("/opt/trn_rl_repo",):
        if os.path.isdir(_p) and _p not in sys.path:
            sys.path.insert(0, _p)

import concourse.bass as bass
import concourse.tile as tile
from concourse import bacc, mybir
from concourse.bass import ds, ts
from concourse.bass_utils import run_bass_kernel_spmd

F16 = mybir.dt.float16
I16 = mybir.dt.int16
F32 = mybir.dt.float32

B, N, CH = 4, 2048, 1024
H, D = 16, 64
NCORES = 8
DC = 512                   # channel dims per core (8 heads)
NP = 4                     # head pairs per core
TBS = 512                  # token block size
NTB = N // TBS             # 4 token blocks
CK = CH // 128             # 8 contraction chunks for QKV projections
KT = N // 128              # 16 key tiles
QB = N // 512              # 4 query blocks

# Schraudolph exp in fp16-bit space: exp(s) ~= bitcast_f16(i16(A*s + B)).
EXPA = float(2.0**10 / np.log(2.0))
EXPB = float(15.0 * 1024.0 - 44.0)

MULT = mybir.AluOpType.mult
ADD = mybir.AluOpType.add
IDENT = mybir.ActivationFunctionType.Identity
EXP = mybir.ActivationFunctionType.Exp

AV_LAG = 2


def build_nc(debug: bool = False):
    nc = bacc.Bacc("TRN2", target_bir_lowering=False, debug=debug)

    xTd = nc.dram_tensor("xTd", [128, NTB, CK, TBS], F16, kind="ExternalInput")
    wq_d = nc.dram_tensor("wq", [128, CK, DC], F16, kind="ExternalInput")
    wk_d = nc.dram_tensor("wk", [128, CK, DC], F16, kind="ExternalInput")
    wv_d = nc.dram_tensor("wv", [128, CK, DC], F16, kind="ExternalInput")
    wo_d = nc.dram_tensor("wo", [128, NP, CH], F16, kind="ExternalInput")
    bqkv_d = nc.dram_tensor("bqkv", [128, NP, 3], F32, kind="ExternalInput")
    out_d = nc.dram_tensor("out_p", [N, CH], F16, kind="ExternalOutput")
    den_d = nc.dram_tensor("den_scr", [QB * NP * 2, 512], F32)

    with tile.TileContext(nc) as tc:
        with tc.tile_pool(name="const", bufs=1) as const:
            wq_sb = const.tile([128, CK, DC], F16, tag="wq")
            wk_sb = const.tile([128, CK, DC], F16, tag="wk")
            wv_sb = const.tile([128, CK, DC], F16, tag="wv")
            wo_sb = const.tile([128, NP, CH], F16, tag="wo")
            bqkv_sb = const.tile([128, NP, 3], F32, tag="bqkv")
            xt = const.tile([128, NTB, CK, TBS], F16, tag="xt")
            QTs = [const.tile([128, N], F16, tag=f"QT{hp}", name=f"QT{hp}")
                   for hp in range(NP)]
            KTs = [const.tile([128, N], F16, tag=f"KT{hp}", name=f"KT{hp}")
                   for hp in range(NP)]
            # VA blocks padded to 128 cols ([V|ones|zeros]) so the AV
            # LDWEIGHTS is a full-128-col load and FWL kicks in
            VAs = [const.tile([128, KT, 256], F16, tag=f"VA{hp}", name=f"VA{hp}")
                   for hp in range(NP)]
            CTs = [const.tile([128, N], F16, tag=f"CT{hp}", name=f"CT{hp}")
                   for hp in range(NP)]

            # weight DMAs on the gpsimd queue, in first-use order (wv feeds
            # the V matmuls that start the kernel); x tiles on the sync queue
            nc.gpsimd.dma_start(out=wv_sb, in_=wv_d[:])
            nc.gpsimd.dma_start(out=wq_sb, in_=wq_d[:])
            nc.gpsimd.dma_start(out=wk_sb, in_=wk_d[:])
            nc.gpsimd.dma_start(out=bqkv_sb, in_=bqkv_d[:])
            nc.gpsimd.dma_start(out=wo_sb, in_=wo_d[:])
            # x tile chunks: ck-PAIRS so every DMA moves 2KB per partition
            # row (the 1KB-row version ran at half throughput)
            for tb in range(NTB):
                for j in range(CK // 2):
                    nc.sync.dma_start(out=xt[:, tb, 2 * j : 2 * j + 2],
                                      in_=xTd[:, tb, 2 * j : 2 * j + 2])
            # zero the VA padding, then ones columns for the softmax
            # denominators (col 64 of each head's 128-col block)
            for hp in range(NP):
                nc.gpsimd.memset(VAs[hp], 0.0)
            for hp in range(NP):
                nc.vector.memset(VAs[hp][:, :, 64], 1.0)
                nc.vector.memset(VAs[hp][:, :, 192], 1.0)

            # ---------------- shared emission machinery ----------------
            step_no = [0]
            deferred = []      # (due_step, thunk), due monotonically ordered
            av_queue = []
            proj_mms = []      # (tt, hp2, half)
            po_holder = [None]
            cur_psout = [None]
            pso_cur = [None]

            def flush_deferred(force=False):
                while deferred and (force or deferred[0][0] <= step_no[0]):
                    deferred.pop(0)[1]()

            def queue_proj(tt):
                for hp2 in range(NP):
                    for half in range(2):
                        proj_mms.append((tt, hp2, half))

            def emit_one_proj_mm():
                tt, hp2, half = proj_mms.pop(0)
                if hp2 == 0 and half == 0:
                    po_holder[0] = cur_psout[0].tile([128, CH], F32,
                                                     tag="po", name="po")
                po = po_holder[0]
                nc.tensor.matmul(po[:, ts(half, 512)], CTs[hp2][:, ts(tt, 128)],
                                 wo_sb[:, hp2, ts(half, 512)],
                                 start=(hp2 == 0), stop=(hp2 == NP - 1))
                if hp2 == NP - 1 and half == 1:
                    ob = ob_pool.tile([128, CH], F16, tag="ob", name="ob")
                    nc.scalar.copy(ob, po)
                    nc.sync.dma_start(out=out_d[ts(tt, 128), :], in_=ob)

            def finish_hp(qb, hp, pa, pb):
                # evacuate both AV banks promptly so the next (qb,hp)'s AV
                # can reuse the PSUM; row 64 of each is the softmax
                # denominator
                cx_a = cx_pool.tile([65, 512], F32, tag="ca", name="cx_a")
                cx_b = cx_pool.tile([65, 512], F32, tag="cb", name="cx_b")
                nc.scalar.copy(cx_a, pa[0:65])
                nc.vector.tensor_copy(cx_b, pb[0:65])
                den2 = nrm_pool.tile([2, 512], F32, tag="den", name="den2")
                nc.gpsimd.dma_start(out=den2[0:1], in_=cx_a[64:65])
                nc.gpsimd.dma_start(out=den2[1:2], in_=cx_b[64:65])
                base = (qb * NP + hp) * 2

                def f2():
                    # reciprocal + broadcast issue, deferred so the den2 DMA
                    # latency never blocks the DVE queue head
                    rec2 = nrm_pool.tile([2, 512], F32, tag="rec", name="rec2")
                    nc.vector.reciprocal_approx_fast(rec2, den2)
                    nc.gpsimd.dma_start(out=den_d[base : base + 2], in_=rec2)
                    rb_a = rb_pool.tile([64, 512], F32, tag="ra", name="rb_a")
                    nc.gpsimd.dma_start(
                        out=rb_a,
                        in_=den_d[base : base + 1].to_broadcast([64, 512]))
                    rb_b = rb_pool.tile([64, 512], F32, tag="rb", name="rb_b")
                    nc.gpsimd.dma_start(
                        out=rb_b,
                        in_=den_d[base + 1 : base + 2].to_broadcast([64, 512]))

                    def f3():
                        qsl = ds(qb * 512, 512)
                        # head-a rows are partition-aligned -> gpsimd (idle
                        # engine); head-b needs a +64 partition shift -> DVE
                        nc.gpsimd.tensor_tensor(CTs[hp][0:64, qsl],
                                                cx_a[0:64], rb_a, MULT)
                        nc.vector.tensor_mul(CTs[hp][64:128, qsl],
                                             cx_b[0:64], rb_b)
                        if hp == NP - 1:
                            for tt in range(qb * 4, qb * 4 + 4):
                                queue_proj(tt)

                    deferred.append((step_no[0] + 3, f3))

                deferred.append((step_no[0] + 3, f2))

            def emit_av(entry):
                pt, qb, hp, kt, pa, pb = entry
                va = VAs[hp]
                nc.tensor.matmul(pa, va[:, kt, 0:128], pt[:, 0],
                                 start=(kt == 0), stop=(kt == KT - 1))
                nc.tensor.matmul(pb, va[:, kt, 128:256], pt[:, 1],
                                 start=(kt == 0), stop=(kt == KT - 1))
                if kt == KT - 1:
                    finish_hp(qb, hp, pa, pb)

            def emit_step(qb, hp, kt):
                if kt == 0:
                    pso_cur[0] = (
                        pso_pool.tile([128, 512], F32, tag="pa", name="pso_a"),
                        pso_pool.tile([128, 512], F32, tag="pb", name="pso_b"))
                qsl = ds(qb * 512, 512)
                ksl = ds(kt * 128, 128)
                # both heads' scores into ONE 2-bank psum tile; the two
                # matmuls are row-tiled (K=64) and run concurrently
                ss = pss_pool.tile([128, 2, TBS], F32, tag="ss", name="ss_pair")
                nc.tensor.matmul(ss[:, 0], KTs[hp][0:64, ksl],
                                 QTs[hp][0:64, qsl], start=True, stop=True)
                nc.tensor.matmul(ss[:, 1], KTs[hp][64:128, ksl],
                                 QTs[hp][64:128, qsl], start=True, stop=True)
                pt = pt_pool.tile([128, 2, TBS], F16, tag="pt", name="pt_pair")
                # ONE merged exp instruction for both heads, alternating
                # engines: even kt -> DVE Schraudolph, odd kt -> ACT exact.
                # Same 50% exact / 50% approx key mix as before.
                if kt % 2 == 0:
                    nc.vector.tensor_scalar(pt.bitcast(I16), ss,
                                            EXPA, EXPB, MULT, ADD)
                else:
                    nc.scalar.activation(pt, ss, EXP)
                av_queue.append((pt, qb, hp, kt) + pso_cur[0])

            def make_qk_thunks(hp):
                thunks = []
                state = {}
                for tb in range(NTB):
                    for w_sb, dst, bcol in ((wq_sb, QTs[hp], 0),
                                            (wk_sb, KTs[hp], 1)):
                        for ck in range(CK):
                            def th(tb=tb, w_sb=w_sb, dst=dst, bcol=bcol, ck=ck):
                                if ck == 0:
                                    state["ps"] = qk_pool.tile(
                                        [128, TBS], F32, tag="qk",
                                        name=f"psqk{hp}")
                                ps = state["ps"]
                                nc.tensor.matmul(
                                    ps, w_sb[:, ck, ds(hp * 128, 128)],
                                    xt[:, tb, ck],
                                    start=(ck == 0), stop=(ck == CK - 1))
                                if ck == CK - 1:
                                    nc.scalar.activation(
                                        dst[:, ts(tb, TBS)], ps, IDENT,
                                        bias=bqkv_sb[:, hp, bcol : bcol + 1])
                            thunks.append(th)
                return thunks

            def window(hp, drip):
                for qb in range(QB):
                    for kt in range(KT):
                        emit_step(qb, hp, kt)
                        step_no[0] += 1
                        flush_deferred()
                        while len(av_queue) > AV_LAG:
                            emit_av(av_queue.pop(0))
                        if drip:
                            drip.pop(0)()
                        elif hp == NP - 1:
                            budget = 3 if len(proj_mms) > 40 else 2
                            for _ in range(budget):
                                if proj_mms:
                                    emit_one_proj_mm()
                while av_queue:
                    emit_av(av_queue.pop(0))
                while drip:
                    drip.pop(0)()

            # ---------------- emission ----------------
            with tc.tile_pool(name="pt", bufs=2 + AV_LAG) as pt_pool, \
                 tc.tile_pool(name="cx", bufs=2) as cx_pool, \
                 tc.tile_pool(name="nrm", bufs=2) as nrm_pool, \
                 tc.tile_pool(name="rb", bufs=2) as rb_pool, \
                 tc.tile_pool(name="ob", bufs=3) as ob_pool:

              with tc.tile_pool(name="ps_s", bufs=2, space="PSUM") as pss_pool, \
                   tc.tile_pool(name="ps_av", bufs=1, space="PSUM") as pso_pool:

                with tc.tile_pool(name="ps_qk", bufs=2, space="PSUM") as qk_pool:
                    # lead-in 1: V for all tokens (psv shares the qk ring)
                    for tb in range(NTB):
                        for i in range(TBS // 128):
                            g = tb * (TBS // 128) + i
                            psv = qk_pool.tile([128, DC], F32, tag="qk",
                                               name="psv")
                            for ck in range(CK):
                                nc.tensor.matmul(psv, xt[:, tb, ck, ts(i, 128)],
                                                 wv_sb[:, ck, :],
                                                 start=(ck == 0),
                                                 stop=(ck == CK - 1))
                            for hp in range(NP):
                                eng = nc.vector if hp < 2 else nc.scalar
                                if hp < 2:
                                    eng.tensor_copy(VAs[hp][:, g, 0:64],
                                                    psv[:, ds(hp * 128, 64)])
                                    eng.tensor_copy(VAs[hp][:, g, 128:192],
                                                    psv[:, ds(hp * 128 + 64, 64)])
                                else:
                                    eng.copy(VAs[hp][:, g, 0:64],
                                             psv[:, ds(hp * 128, 64)])
                                    eng.copy(VAs[hp][:, g, 128:192],
                                             psv[:, ds(hp * 128 + 64, 64)])
                    # lead-in 2: Q/K for head pair 0
                    for th in make_qk_thunks(0):
                        th()
                    # attention windows 0..2, dripping the next pair's Q/K
                    for hp in range(NP - 1):
                        window(hp, make_qk_thunks(hp + 1))
                # last window: drip the output projection instead
                with tc.tile_pool(name="ps_out", bufs=1, space="PSUM") as po_p:
                    cur_psout[0] = po_p
                    window(NP - 1, None)
                    flush_deferred(force=True)
                    # drain to a po-group boundary inside this pool
                    while proj_mms and not (proj_mms[0][1] == 0
                                            and proj_mms[0][2] == 0):
                        emit_one_proj_mm()
              # tail: remaining projections with a deeper PSUM ring (the
              # attention pools above are closed, freeing their banks)
              with tc.tile_pool(name="ps_tail", bufs=2, space="PSUM") as ptail:
                  cur_psout[0] = ptail
                  while proj_mms:
                      emit_one_proj_mm()

    nc.compile()
    return nc


def make_in_maps(x, Wq, bq, Wk, bk, Wv, bv, Wo, bo):
    """Host-side sharding: per-core input dict (all numpy, fp16)."""
    scale = D ** -0.5
    F16N = np.float16
    xf = np.asarray(x, np.float32)
    Wqs = np.asarray(Wq, np.float32) * scale
    bqs = np.asarray(bq, np.float32) * scale

    in_maps = []
    for c in range(NCORES):
        b, hh = c >> 1, c & 1
        cols = slice(hh * DC, (hh + 1) * DC)
        xb = xf[b]  # [N, CH]
        xT = np.ascontiguousarray(
            xb.reshape(NTB, TBS, CK, 128).transpose(3, 0, 2, 1)
        ).astype(F16N)

        def wsl(W):
            Wc = np.asarray(W, np.float32)[:, cols]
            return np.ascontiguousarray(
                Wc.reshape(CK, 128, DC).transpose(1, 0, 2)).astype(F16N)

        wo_c = np.asarray(Wo, np.float32)[cols, :]
        wo_c = np.ascontiguousarray(
            wo_c.reshape(NP, 128, CH).transpose(1, 0, 2)).astype(F16N)
        bqkv = np.stack(
            [bqs[cols], np.asarray(bk, np.float32)[cols],
             np.asarray(bv, np.float32)[cols]], axis=1,
        ).astype(np.float32).reshape(NP, 128, 3).transpose(1, 0, 2)
        in_maps.append({
            "xTd": xT,
            "wq": wsl(Wqs),
            "wk": wsl(Wk),
            "wv": wsl(Wv),
            "wo": wo_c,
            "bqkv": np.ascontiguousarray(bqkv),
        })
    return in_maps


_NC_CACHE = {}


def get_nc(debug: bool = False):
    if debug not in _NC_CACHE:
        _NC_CACHE[debug] = build_nc(debug=debug)
    return _NC_CACHE[debug]


def kernel(x, Wq, bq, Wk, bk, Wv, bv, Wo, bo, _trace=False):
    nc = get_nc()
    in_maps = make_in_maps(x, Wq, bq, Wk, bk, Wv, bv, Wo, bo)
    res = run_bass_kernel_spmd(nc, in_maps, list(range(NCORES)), trace=_trace)
    out = np.zeros((B, N, CH), np.float32)
    for c, r in enumerate(res.results):
        out[c >> 1] += np.asarray(r["out_p"], np.float32)
    # bv contributes bv @ Wo to every token (softmax weights sum to 1), so it
    # folds into the output bias on the host
    bias = np.asarray(bo, np.float32) + np.asarray(bv, np.float32) @ np.asarray(Wo, np.float32)
    out += bias[None, None, :]
    if _trace:
        return out, res
    return out


# revision 11
# speedup vs baseline: 1.0120x; 1.0120x over previous
"""Fused multi-head attention (B=4, N=2048, C=1024, H=16) for 8 trn2 NeuronCores.

Sharding: batch x head-half hybrid. Core c owns batch b = c>>1 and head-half
hh = c&1 (8 heads = channel dims hh*512..hh*512+512, as 4 head-pairs). Each
core computes QKV for its batch restricted to its 512 dims, attention for its
8 heads, and the partial output projection [2048, 1024] for its batch; the
host sums the 2 partials per batch and adds bo.

Head-pair-major fused schedule (v2): the old kernel ran QKV fully (PE-dense,
ACT/DVE idle) then attention (exp-bound on ACT/DVE, PE ~15% idle). Now only
V + the first head-pair's Q/K run up front; each attention window for head
pair hp drips the NEXT pair's Q/K projection matmuls (one per step) into the
PE slack that the exp latency leaves, and the last window drips the output
projection. This hides ~55us of projection work inside the exp-bound phase.

Per step (qb, hp, kt): the two heads' score matmuls run as row-tiled
CONCURRENT matmuls (K=64 each, tile_position (0,0)/(64,0)) into one
2-bank PSUM tile [128, 2, 512]; ONE merged exp instruction covers both
heads (alternating: even kt -> DVE Schraudolph fp16-bit exp, odd kt -> ACT
exact exp), halving per-instruction overheads; AV trails by AV_LAG steps.
Softmax denominators come from ones-columns in the packed V blocks
([V_h0|ones|V_h1|ones] per k-tile). Normalization (reciprocal + DMA
broadcast of 1/den + multiply into CT) is emitted DEFERRED by a few steps
so the DMA-roundtrip latency never head-of-line-blocks the DVE/ACT queues.
"""

import os
import sys

import numpy as np

if not os.path.isdir(os.path.join(os.path.dirname(os.path.abspath(__file__)), "concourse")):
    for _p in ("/opt/trn_rl_repo",):
        if os.path.isdir(_p) and _p not in sys.path:
            sys.path.insert(0, _p)

import concourse.bass as bass
import concourse.tile as tile
from concourse import bacc, mybir
from concourse.bass import ds, ts
from concourse.bass_utils import run_bass_kernel_spmd

F16 = mybir.dt.float16
I16 = mybir.dt.int16
F32 = mybir.dt.float32

B, N, CH = 4, 2048, 1024
H, D = 16, 64
NCORES = 8
DC = 512                   # channel dims per core (8 heads)
NP = 4                     # head pairs per core
TBS = 512                  # token block size
NTB = N // TBS             # 4 token blocks
CK = CH // 128             # 8 contraction chunks for QKV projections
KT = N // 128              # 16 key tiles
QB = N // 512              # 4 query blocks

# Schraudolph exp in fp16-bit space: exp(s) ~= bitcast_f16(i16(A*s + B)).
EXPA = float(2.0**10 / np.log(2.0))
EXPB = float(15.0 * 1024.0 - 44.0)

MULT = mybir.AluOpType.mult
ADD = mybir.AluOpType.add
IDENT = mybir.ActivationFunctionType.Identity
EXP = mybir.ActivationFunctionType.Exp

AV_LAG = 2


def build_nc(debug: bool = False):
    nc = bacc.Bacc("TRN2", target_bir_lowering=False, debug=debug)

    xTd = nc.dram_tensor("xTd", [128, NTB, CK, TBS], F16, kind="ExternalInput")
    wq_d = nc.dram_tensor("wq", [128, CK, DC], F16, kind="ExternalInput")
    wk_d = nc.dram_tensor("wk", [128, CK, DC], F16, kind="ExternalInput")
    wv_d = nc.dram_tensor("wv", [128, CK, DC], F16, kind="ExternalInput")
    wo_d = nc.dram_tensor("wo", [128, NP, CH], F16, kind="ExternalInput")
    bqkv_d = nc.dram_tensor("bqkv", [128, NP, 3], F32, kind="ExternalInput")
    out_d = nc.dram_tensor("out_p", [N, CH], F16, kind="ExternalOutput")
    den_d = nc.dram_tensor("den_scr", [QB * NP * 2, 512], F16)

    with tile.TileContext(nc) as tc:
        with tc.tile_pool(name="const", bufs=1) as const:
            wq_sb = const.tile([128, CK, DC], F16, tag="wq")
            wk_sb = const.tile([128, CK, DC], F16, tag="wk")
            wv_sb = const.tile([128, CK, DC], F16, tag="wv")
            wo_sb = const.tile([128, NP, CH], F16, tag="wo")
            bqkv_sb = const.tile([128, NP, 3], F32, tag="bqkv")
            xt = const.tile([128, NTB, CK, TBS], F16, tag="xt")
            QTs = [const.tile([128, N], F16, tag=f"QT{hp}", name=f"QT{hp}")
                   for hp in range(NP)]
            KTs = [const.tile([128, N], F16, tag=f"KT{hp}", name=f"KT{hp}")
                   for hp in range(NP)]
            # VA blocks padded to 128 cols ([V|ones|zeros]) so the AV
            # LDWEIGHTS is a full-128-col load and FWL kicks in
            VAs = [const.tile([128, KT, 256], F16, tag=f"VA{hp}", name=f"VA{hp}")
                   for hp in range(NP)]
            CTs = [const.tile([128, N], F16, tag=f"CT{hp}", name=f"CT{hp}")
                   for hp in range(NP)]

            # weight DMAs on the gpsimd queue, in first-use order (wv feeds
            # the V matmuls that start the kernel); x tiles on the sync queue
            nc.gpsimd.dma_start(out=wv_sb, in_=wv_d[:])
            nc.gpsimd.dma_start(out=wq_sb, in_=wq_d[:])
            nc.gpsimd.dma_start(out=wk_sb, in_=wk_d[:])
            nc.gpsimd.dma_start(out=bqkv_sb, in_=bqkv_d[:])
            nc.gpsimd.dma_start(out=wo_sb, in_=wo_d[:])
            # x tiles: one whole-tb DMA each (8KB per partition row — the
            # chunked 1-2KB-row versions ran at a fraction of peak and
            # serialized 16 issue instructions on the sync queue)
            for tb in range(NTB):
                nc.sync.dma_start(out=xt[:, tb], in_=xTd[:, tb])
            # zero the VA padding, then ones columns for the softmax
            # denominators (col 64 of each head's 128-col block)
            for hp in range(NP):
                nc.gpsimd.memset(VAs[hp], 0.0)
            for hp in range(NP):
                nc.vector.memset(VAs[hp][:, :, 64], 1.0)
                nc.vector.memset(VAs[hp][:, :, 192], 1.0)

            # ---------------- shared emission machinery ----------------
            step_no = [0]
            deferred = []      # (due_step, thunk), due monotonically ordered
            av_queue = []
            proj_mms = []      # (tt, hp2, half)
            po_holder = [None]
            cur_psout = [None]
            pso_cur = [None]

            def flush_deferred(force=False):
                while deferred and (force or deferred[0][0] <= step_no[0]):
                    deferred.pop(0)[1]()

            def queue_proj(tt):
                for hp2 in range(NP):
                    for half in range(2):
                        proj_mms.append((tt, hp2, half))

            def emit_one_proj_mm():
                tt, hp2, half = proj_mms.pop(0)
                if hp2 == 0 and half == 0:
                    po_holder[0] = cur_psout[0].tile([128, CH], F32,
                                                     tag="po", name="po")
                po = po_holder[0]
                nc.tensor.matmul(po[:, ts(half, 512)], CTs[hp2][:, ts(tt, 128)],
                                 wo_sb[:, hp2, ts(half, 512)],
                                 start=(hp2 == 0), stop=(hp2 == NP - 1))
                if hp2 == NP - 1 and half == 1:
                    ob = ob_pool.tile([128, CH], F16, tag="ob", name="ob")
                    # alternate the evac engine: ACT also carries the per-step
                    # exact exp, DVE the Schraudolph one — neither has room
                    # for all 16 [128,1024] copies on top
                    if tt % 2 == 0:
                        nc.scalar.copy(ob, po)
                    else:
                        nc.vector.tensor_copy(ob, po)
                    nc.sync.dma_start(out=out_d[ts(tt, 128), :], in_=ob)

            def finish_hp(qb, hp, pa, pb):
                # evacuate both AV banks promptly so the next (qb,hp)'s AV
                # can reuse the PSUM; row 64 of each is the softmax
                # denominator
                cx_a = cx_pool.tile([65, 512], F32, tag="ca", name="cx_a")
                cx_b = cx_pool.tile([65, 512], F32, tag="cb", name="cx_b")
                nc.scalar.copy(cx_a, pa[0:65])
                nc.vector.tensor_copy(cx_b, pb[0:65])
                den2 = nrm_pool.tile([2, 512], F32, tag="den", name="den2")
                nc.gpsimd.dma_start(out=den2[0:1], in_=cx_a[64:65])
                nc.gpsimd.dma_start(out=den2[1:2], in_=cx_b[64:65])
                base = (qb * NP + hp) * 2

                def f2():
                    # reciprocal + broadcast issue, deferred so the den2 DMA
                    # latency never blocks the DVE queue head. Broadcasts in
                    # fp16 (1/den needs ~1e-3 rel err at most): halves the
                    # ~8MB of broadcast DMA write traffic
                    rec2 = nrm_pool.tile([2, 512], F32, tag="rec", name="rec2")
                    nc.vector.reciprocal_approx_fast(rec2, den2)
                    rec2h = nrm_pool.tile([2, 512], F16, tag="rech", name="rec2h")
                    nc.vector.tensor_copy(rec2h, rec2)
                    nc.gpsimd.dma_start(out=den_d[base : base + 2], in_=rec2h)
                    rb_a = rb_pool.tile([64, 512], F16, tag="ra", name="rb_a")
                    nc.gpsimd.dma_start(
                        out=rb_a,
                        in_=den_d[base : base + 1].to_broadcast([64, 512]))
                    rb_b = rb_pool.tile([64, 512], F16, tag="rb", name="rb_b")
                    nc.gpsimd.dma_start(
                        out=rb_b,
                        in_=den_d[base + 1 : base + 2].to_broadcast([64, 512]))

                    def f3():
                        qsl = ds(qb * 512, 512)
                        # head-a rows are partition-aligned -> gpsimd (idle
                        # engine); head-b needs a +64 partition shift -> DVE
                        nc.gpsimd.tensor_tensor(CTs[hp][0:64, qsl],
                                                cx_a[0:64], rb_a, MULT)
                        nc.vector.tensor_mul(CTs[hp][64:128, qsl],
                                             cx_b[0:64], rb_b)
                        if hp == NP - 1:
                            for tt in range(qb * 4, qb * 4 + 4):
                                queue_proj(tt)

                    deferred.append((step_no[0] + 3, f3))

                deferred.append((step_no[0] + 3, f2))

            def emit_av(entry):
                pt, qb, hp, kt, pa, pb = entry
                va = VAs[hp]
                nc.tensor.matmul(pa, va[:, kt, 0:128], pt[:, 0],
                                 start=(kt == 0), stop=(kt == KT - 1))
                nc.tensor.matmul(pb, va[:, kt, 128:256], pt[:, 1],
                                 start=(kt == 0), stop=(kt == KT - 1))
                if kt == KT - 1:
                    finish_hp(qb, hp, pa, pb)

            def emit_step(qb, hp, kt):
                if kt == 0:
                    pso_cur[0] = (
                        pso_pool.tile([128, 512], F32, tag="pa", name="pso_a"),
                        pso_pool.tile([128, 512], F32, tag="pb", name="pso_b"))
                qsl = ds(qb * 512, 512)
                ksl = ds(kt * 128, 128)
                # per-head score tiles (one PSUM bank each) so the WAR chain
                # scores(kt) -> exp(kt) -> scores(kt+2) stays short: a merged
                # 2-bank exp (1130-1250ns) blew the 2-step budget and
                # inflated the step period ~25%
                ss_a = pss_pool.tile([128, TBS], F32, tag="sa", name="ss_a")
                ss_b = pss_pool.tile([128, TBS], F32, tag="sb", name="ss_b")
                # scores for both heads of the pair: concurrent row-tiled
                # matmuls (K=64 each, tile_position (0,0)/(64,0))
                nc.tensor.matmul(ss_a, KTs[hp][0:64, ksl],
                                 QTs[hp][0:64, qsl], start=True, stop=True)
                nc.tensor.matmul(ss_b, KTs[hp][64:128, ksl],
                                 QTs[hp][64:128, qsl], start=True, stop=True)
                pt = pt_pool.tile([128, 2, TBS], F16, tag="pt", name="pt_pair")
                # head-a: exact exp on ACT; head-b: Schraudolph exp on DVE
                nc.scalar.activation(pt[:, 0], ss_a, EXP)
                nc.vector.tensor_scalar(pt[:, 1].bitcast(I16), ss_b,
                                        EXPA, EXPB, MULT, ADD)
                av_queue.append((pt, qb, hp, kt) + pso_cur[0])

            def make_qk_thunks(hp):
                # ALL of K first: the next window's qb0 reads every KT token
                # tile within its first 16 steps, but only QT[:, qb0]; K
                # emitted last caused 1-3us stalls at each window boundary
                thunks = []
                state = {}
                for w_sb, dst, bcol in ((wk_sb, KTs[hp], 1),
                                        (wq_sb, QTs[hp], 0)):
                    for tb in range(NTB):
                        for ck in range(CK):
                            def th(tb=tb, w_sb=w_sb, dst=dst, bcol=bcol, ck=ck):
                                if ck == 0:
                                    state["ps"] = qk_pool.tile(
                                        [128, TBS], F32, tag="qk",
                                        name=f"psqk{hp}")
                                ps = state["ps"]
                                nc.tensor.matmul(
                                    ps, w_sb[:, ck, ds(hp * 128, 128)],
                                    xt[:, tb, ck],
                                    start=(ck == 0), stop=(ck == CK - 1))
                                if ck == CK - 1:
                                    nc.scalar.activation(
                                        dst[:, ts(tb, TBS)], ps, IDENT,
                                        bias=bqkv_sb[:, hp, bcol : bcol + 1])
                            thunks.append(th)
                return thunks

            def window(hp, drip):
                for qb in range(QB):
                    for kt in range(KT):
                        emit_step(qb, hp, kt)
                        step_no[0] += 1
                        flush_deferred()
                        while len(av_queue) > AV_LAG:
                            emit_av(av_queue.pop(0))
                        if drip:
                            drip.pop(0)()
                        elif hp == NP - 1:
                            # drip projections, but hold 16 (2 po-groups) in
                            # reserve: they bridge the final normalize
                            # chain's DMA latency at the tail
                            budget = 3 if len(proj_mms) > 56 else 2
                            for _ in range(budget):
                                if len(proj_mms) > 16:
                                    emit_one_proj_mm()
                while av_queue:
                    emit_av(av_queue.pop(0))
                while drip:
                    drip.pop(0)()

            # ---------------- emission ----------------
            with tc.tile_pool(name="pt", bufs=2 + AV_LAG) as pt_pool, \
                 tc.tile_pool(name="cx", bufs=2) as cx_pool, \
                 tc.tile_pool(name="nrm", bufs=2) as nrm_pool, \
                 tc.tile_pool(name="rb", bufs=2) as rb_pool, \
                 tc.tile_pool(name="ob", bufs=3) as ob_pool:

              with tc.tile_pool(name="ps_s", bufs=2, space="PSUM") as pss_pool, \
                   tc.tile_pool(name="ps_av", bufs=1, space="PSUM") as pso_pool:

                with tc.tile_pool(name="ps_qk", bufs=2, space="PSUM") as qk_pool:
                    # lead-in 1: V for all tokens (psv shares the qk ring)
                    for tb in range(NTB):
                        for i in range(TBS // 128):
                            g = tb * (TBS // 128) + i
                            psv = qk_pool.tile([128, DC], F32, tag="qk",
                                               name="psv")
                            for ck in range(CK):
                                nc.tensor.matmul(psv, xt[:, tb, ck, ts(i, 128)],
                                                 wv_sb[:, ck, :],
                                                 start=(ck == 0),
                                                 stop=(ck == CK - 1))
                            for hp in range(NP):
                                eng = nc.vector if hp < 2 else nc.scalar
                                if hp < 2:
                                    eng.tensor_copy(VAs[hp][:, g, 0:64],
                                                    psv[:, ds(hp * 128, 64)])
                                    eng.tensor_copy(VAs[hp][:, g, 128:192],
                                                    psv[:, ds(hp * 128 + 64, 64)])
                                else:
                                    eng.copy(VAs[hp][:, g, 0:64],
                                             psv[:, ds(hp * 128, 64)])
                                    eng.copy(VAs[hp][:, g, 128:192],
                                             psv[:, ds(hp * 128 + 64, 64)])
                    # lead-in 2: Q/K for head pair 0
                    for th in make_qk_thunks(0):
                        th()
                    # attention windows 0..2, dripping the next pair's Q/K
                    for hp in range(NP - 1):
                        window(hp, make_qk_thunks(hp + 1))
                # last window: drip the output projection instead
                with tc.tile_pool(name="ps_out", bufs=1, space="PSUM") as po_p:
                    cur_psout[0] = po_p
                    window(NP - 1, None)
                    flush_deferred(force=True)
                    # drain to a po-group boundary inside this pool
                    while proj_mms and not (proj_mms[0][1] == 0
                                            and proj_mms[0][2] == 0):
                        emit_one_proj_mm()
              # tail: remaining projections with a deeper PSUM ring (the
              # attention pools above are closed, freeing their banks)
              with tc.tile_pool(name="ps_tail", bufs=2, space="PSUM") as ptail:
                  cur_psout[0] = ptail
                  while proj_mms:
                      emit_one_proj_mm()

    nc.compile()
    return nc


def make_in_maps(x, Wq, bq, Wk, bk, Wv, bv, Wo, bo):
    """Host-side sharding: per-core input dict (all numpy, fp16)."""
    scale = D ** -0.5
    F16N = np.float16
    xf = np.asarray(x, np.float32)
    Wqs = np.asarray(Wq, np.float32) * scale
    bqs = np.asarray(bq, np.float32) * scale

    in_maps = []
    for c in range(NCORES):
        b, hh = c >> 1, c & 1
        cols = slice(hh * DC, (hh + 1) * DC)
        xb = xf[b]  # [N, CH]
        xT = np.ascontiguousarray(
            xb.reshape(NTB, TBS, CK, 128).transpose(3, 0, 2, 1)
        ).astype(F16N)

        def wsl(W):
            Wc = np.asarray(W, np.float32)[:, cols]
            return np.ascontiguousarray(
                Wc.reshape(CK, 128, DC).transpose(1, 0, 2)).astype(F16N)

        wo_c = np.asarray(Wo, np.float32)[cols, :]
        wo_c = np.ascontiguousarray(
            wo_c.reshape(NP, 128, CH).transpose(1, 0, 2)).astype(F16N)
        bqkv = np.stack(
            [bqs[cols], np.asarray(bk, np.float32)[cols],
             np.asarray(bv, np.float32)[cols]], axis=1,
        ).astype(np.float32).reshape(NP, 128, 3).transpose(1, 0, 2)
        in_maps.append({
            "xTd": xT,
            "wq": wsl(Wqs),
            "wk": wsl(Wk),
            "wv": wsl(Wv),
            "wo": wo_c,
            "bqkv": np.ascontiguousarray(bqkv),
        })
    return in_maps


_NC_CACHE = {}


def get_nc(debug: bool = False):
    if debug not in _NC_CACHE:
        _NC_CACHE[debug] = build_nc(debug=debug)
    return _NC_CACHE[debug]


def kernel(x, Wq, bq, Wk, bk, Wv, bv, Wo, bo, _trace=False):
    nc = get_nc()
    in_maps = make_in_maps(x, Wq, bq, Wk, bk, Wv, bv, Wo, bo)
    res = run_bass_kernel_spmd(nc, in_maps, list(range(NCORES)), trace=_trace)
    out = np.zeros((B, N, CH), np.float32)
    for c, r in enumerate(res.results):
        out[c >> 1] += np.asarray(r["out_p"], np.float32)
    # bv contributes bv @ Wo to every token (softmax weights sum to 1), so it
    # folds into the output bias on the host
    bias = np.asarray(bo, np.float32) + np.asarray(bv, np.float32) @ np.asarray(Wo, np.float32)
    out += bias[None, None, :]
    if _trace:
        return out, res
    return out
